# revision 7
# baseline (speedup 1.0000x reference)
"""AdaptiveExpertRouter Trainium2 kernel (8 NeuronCores, data-parallel over tokens).

Per-core pipeline (512 tokens, feature-major activations [feat_part, tok_free]):
  - every Linear is a 3-term bf16 hi/lo split matmul (x@W = xh@Wh + xh@Wl + xl@Wh)
    accumulated in fp32 PSUM -> ~7e-6 relative error, needed so top-2 expert
    selection matches the fp32 reference bit-for-bit on realistic score gaps
  - LayerNorm stats via ones-row matmuls (mean from bf16 hi/lo, E[x^2] from an
    fp32r Square), per-token broadcast back via K=1 ones matmul
  - logits transposed to token-major with PE transpose, softmax + top-2 via the
    DVE max8/max_index ops, then an indirect-DMA gather of the 2 selected
    expert rows per token and an fp32 weighted combine.
"""

import os
import sys
from contextlib import ExitStack

for _p in ("/opt/trn_rl_repo", "/root/.axon_site/_ro/trn_rl_repo"):
    if os.path.isdir(_p) and _p not in sys.path:
        sys.path.append(_p)

import numpy as np
import ml_dtypes

import concourse.bass as bass
import concourse.mybir as mybir
import concourse.bacc as bacc
from concourse.tile import TileContext
from concourse.bass_utils import run_bass_kernel_spmd
from concourse.masks import make_identity

P = 128
NCORES = 8
NTOK = 4096          # B*S
CTOK = NTOK // NCORES  # 512 tokens per core
NTT = CTOK // P      # 4 token tiles per core
S_DIM = 1024
T_DIM = 4096
E = 8

f32 = mybir.dt.float32
f32r = mybir.dt.float32r
bf16 = mybir.dt.bfloat16
i32 = mybir.dt.int32
u32 = mybir.dt.uint32

bf = ml_dtypes.bfloat16

# layer name -> (in_dim, out_dim, has_ln, has_gelu)
LAYERS = {
    "t2s":  (T_DIM, S_DIM),
    "es1":  (T_DIM, 2 * S_DIM),
    "es2":  (2 * S_DIM, P),      # logits padded 8 -> 128
    "cap1": (S_DIM, 2 * S_DIM),
    "cap2": (2 * S_DIM, S_DIM),
    "cap3": (S_DIM, P),          # logits padded
    "gap1": (2 * S_DIM, T_DIM),
    "gap2": (T_DIM, P),          # logits padded
}
LN_LAYERS = {"cap1", "cap2", "gap1"}


def _build(inv_temp: float):
    nc = bacc.Bacc("TRN2", target_bir_lowering=False, debug=False,
                   num_devices=NCORES)

    ext = {}
    ext["st_h"] = nc.dram_tensor("st_h", [S_DIM, CTOK], bf16, kind="ExternalInput")
    ext["st_l"] = nc.dram_tensor("st_l", [S_DIM, CTOK], bf16, kind="ExternalInput")
    ext["te_h"] = nc.dram_tensor("te_h", [T_DIM, CTOK], bf16, kind="ExternalInput")
    ext["te_l"] = nc.dram_tensor("te_l", [T_DIM, CTOK], bf16, kind="ExternalInput")
    for L, (ind, outd) in LAYERS.items():
        ext[f"{L}_wh"] = nc.dram_tensor(f"{L}_wh", [ind, outd], bf16, kind="ExternalInput")
        ext[f"{L}_wl"] = nc.dram_tensor(f"{L}_wl", [ind, outd], bf16, kind="ExternalInput")
        ext[f"{L}_b"] = nc.dram_tensor(f"{L}_b", [outd, 1], f32, kind="ExternalInput")
        if L in LN_LAYERS:
            ext[f"{L}_g"] = nc.dram_tensor(f"{L}_g", [outd, 1], f32, kind="ExternalInput")
            ext[f"{L}_be"] = nc.dram_tensor(f"{L}_be", [outd, 1], f32, kind="ExternalInput")
    ext["iota"] = nc.dram_tensor("iota", [P, 1], f32, kind="ExternalInput")
    ext["eo"] = nc.dram_tensor("eo", [E * CTOK, T_DIM], f32, kind="ExternalInput")
    out_ext = nc.dram_tensor("out", [CTOK, T_DIM], f32, kind="ExternalOutput")

    with TileContext(nc) as tc, ExitStack() as top:
        const = top.enter_context(tc.tile_pool(name="const", bufs=1))
        ident = const.tile([P, P], f32, name="ident")
        make_identity(nc, ident)
        iota_sb = const.tile([P, 1], f32, name="iota_sb")
        nc.sync.dma_start(out=iota_sb[:], in_=ext["iota"][:])
        eps_t = const.tile([1, 1], f32, name="eps_t")
        nc.vector.memset(eps_t[:], 1e-5)
        ones1 = const.tile([1, P], f32, name="ones1")
        nc.vector.memset(ones1[:], 1.0)
        # 1/D ones rows for stats matmuls (exact powers of two)
        ones_bf = {}
        ones_fr = {}
        for D in (S_DIM, 2 * S_DIM, T_DIM):
            tb = const.tile([P, 1], bf16, name=f"ones_bf_{D}")
            nc.vector.memset(tb[:], 1.0 / D)
            ones_bf[D] = tb
            t0 = const.tile([P, 1], f32, name=f"ones_f_{D}")
            nc.vector.memset(t0[:], 1.0 / D)
            tr = const.tile([P, 1], f32r, name=f"ones_fr_{D}")
            nc.vector.tensor_copy(out=tr[:], in_=t0[:])
            ones_fr[D] = tr

        biasp = top.enter_context(tc.tile_pool(name="biasp", bufs=1))
        wpool = top.enter_context(tc.tile_pool(name="wpool", bufs=6))
        lnt = top.enter_context(tc.tile_pool(name="lnt", bufs=8))
        statp = top.enter_context(tc.tile_pool(name="statp", bufs=4))
        bcp = top.enter_context(tc.tile_pool(name="bcp", bufs=2))
        mm_ctx = ExitStack()
        psA = mm_ctx.enter_context(tc.tile_pool(name="psA", bufs=5, space="PSUM"))
        psS = mm_ctx.enter_context(tc.tile_pool(name="psS", bufs=2, space="PSUM"))
        psB = mm_ctx.enter_context(tc.tile_pool(name="psB", bufs=1, space="PSUM"))

        def load_vec(name, outd):
            """[outd,1] fp32 dram -> sbuf tile"""
            t = biasp.tile([P, outd // P, 1], f32, name=f"{name}_sb")
            nc.sync.dma_start(
                out=t[:], in_=ext[name][:].rearrange("(ot p) one -> p ot one", p=P))
            return t

        def load_acts(pool, nm, src_h, src_l, dim):
            kt = dim // P
            h = pool.tile([P, kt, CTOK], bf16, name=f"{nm}_h")
            l = pool.tile([P, kt, CTOK], bf16, name=f"{nm}_l")
            nc.sync.dma_start(out=h[:], in_=src_h[:].rearrange("(kt p) n -> p kt n", p=P))
            nc.sync.dma_start(out=l[:], in_=src_l[:].rearrange("(kt p) n -> p kt n", p=P))
            return [(h[:, k, :], l[:, k, :]) for k in range(kt)]

        def split_linear(L, rhs_pairs, epilogue):
            """Emit the 3-term split matmuls for layer L; epilogue(ot, psum_ap)
            is called once per 128-row output tile."""
            ind, outd = LAYERS[L]
            nkt = ind // P
            assert len(rhs_pairs) == nkt
            n_ot = outd // P
            for og in range(0, n_ot, 4):
                ots = list(range(og, min(og + 4, n_ot)))
                ps = {}
                for ot in ots:
                    ps[ot] = psA.tile([P, CTOK], f32, name=f"{L}_ps{ot}", tag="psA")
                for kt in range(nkt):
                    cs = slice(ots[0] * P, (ots[-1] + 1) * P)
                    wh = wpool.tile([P, len(ots) * P], bf16, name=f"{L}_wh{og}_{kt}", tag="wblk")
                    wl = wpool.tile([P, len(ots) * P], bf16, name=f"{L}_wl{og}_{kt}", tag="wblk")
                    nc.sync.dma_start(out=wh[:], in_=ext[f"{L}_wh"][kt * P:(kt + 1) * P, cs])
                    nc.sync.dma_start(out=wl[:], in_=ext[f"{L}_wl"][kt * P:(kt + 1) * P, cs])
                    xh, xl = rhs_pairs[kt]
                    first = kt == 0
                    last = kt == nkt - 1
                    for j, ot in enumerate(ots):
                        sl = slice(j * P, (j + 1) * P)
                        nc.tensor.matmul(ps[ot][:], lhsT=wh[:, sl], rhs=xh,
                                         start=first, stop=False)
                        nc.tensor.matmul(ps[ot][:], lhsT=wl[:, sl], rhs=xh,
                                         start=False, stop=False)
                        nc.tensor.matmul(ps[ot][:], lhsT=wh[:, sl], rhs=xl,
                                         start=False, stop=last)
                for ot in ots:
                    epilogue(ot, ps[ot][:])

        def plain_split_layer(L, rhs_pairs, pool, gelu):
            """Linear (+bias) [+gelu], output split to bf16 hi/lo pairs."""
            _, outd = LAYERS[L]
            n_ot = outd // P
            b = load_vec(f"{L}_b", outd)
            hs, ls = [], []

            def epi(ot, psum):
                ba = b[:, ot, :]
                h = pool.tile([P, CTOK], bf16, name=f"{L}_h{ot}", tag=f"{L}a")
                l = pool.tile([P, CTOK], bf16, name=f"{L}_l{ot}", tag=f"{L}a")
                if gelu:
                    g32 = lnt.tile([P, CTOK], f32, name=f"{L}_g32_{ot}", tag="lnt")
                    nc.scalar.activation(g32[:], psum,
                                         mybir.ActivationFunctionType.Gelu, bias=ba)
                    nc.scalar.copy(h[:], g32[:])
                    nc.vector.tensor_sub(l[:], g32[:], h[:])
                else:
                    nc.scalar.activation(h[:], psum,
                                         mybir.ActivationFunctionType.Identity, bias=ba)
                    d1 = lnt.tile([P, CTOK], f32, name=f"{L}_d1_{ot}", tag="lnt")
                    nc.vector.tensor_sub(d1[:], psum, h[:])
                    nc.vector.tensor_scalar(l[:], d1[:], ba, None,
                                            op0=mybir.AluOpType.add)
                hs.append(h)
                ls.append(l)

            split_linear(L, rhs_pairs, epi)
            return [(hs[i][:], ls[i][:]) for i in range(n_ot)]

        def ln_gelu_layer(L, rhs_pairs, pool):
            """Linear + bias + LayerNorm(g,be) + exact GELU, split output."""
            _, outd = LAYERS[L]
            n_ot = outd // P
            b = load_vec(f"{L}_b", outd)
            g = load_vec(f"{L}_g", outd)
            be = load_vec(f"{L}_be", outd)
            mu_ps = psS.tile([1, CTOK], f32, name=f"{L}_mu", tag="psS")
            m2_ps = psS.tile([1, CTOK], f32, name=f"{L}_m2", tag="psS")
            yhs, yls = [], []

            def epi(ot, psum):
                ba = b[:, ot, :]
                yh = pool.tile([P, CTOK], bf16, name=f"{L}_yh{ot}", tag=f"{L}a")
                yl = pool.tile([P, CTOK], bf16, name=f"{L}_yl{ot}", tag=f"{L}a")
                nc.scalar.activation(yh[:], psum,
                                     mybir.ActivationFunctionType.Identity, bias=ba)
                d1 = lnt.tile([P, CTOK], f32, name=f"{L}_d1_{ot}", tag="lnt")
                nc.vector.tensor_sub(d1[:], psum, yh[:])
                nc.vector.tensor_scalar(yl[:], d1[:], ba, None,
                                        op0=mybir.AluOpType.add)
                sq = lnt.tile([P, CTOK], f32r, name=f"{L}_sq_{ot}", tag="lnt")
                nc.scalar.activation(sq[:], psum,
                                     mybir.ActivationFunctionType.Square, bias=ba)
                first = ot == 0
                last = ot == n_ot - 1
                nc.tensor.matmul(mu_ps[:], lhsT=ones_bf[outd][:], rhs=yh[:],
                                 start=first, stop=False)
                nc.tensor.matmul(mu_ps[:], lhsT=ones_bf[outd][:], rhs=yl[:],
                                 start=False, stop=last)
                nc.tensor.matmul(m2_ps[:], lhsT=ones_fr[outd][:], rhs=sq[:],
                                 start=first, stop=last)
                yhs.append(yh)
                yls.append(yl)

            split_linear(L, rhs_pairs, epi)

            # finalize stats: var = E[y^2] - mu^2 ; rstd = 1/sqrt(var+eps)
            mu = statp.tile([1, CTOK], f32, name=f"{L}_mu_sb", tag="stat")
            nc.vector.tensor_copy(out=mu[:], in_=mu_ps[:])
            var = statp.tile([1, CTOK], f32, name=f"{L}_var", tag="stat")
            nc.vector.tensor_mul(var[:], mu[:], mu[:])
            nc.vector.tensor_sub(var[:], m2_ps[:], var[:])
            std = statp.tile([1, CTOK], f32, name=f"{L}_std", tag="stat")
            nc.scalar.activation(std[:], var[:],
                                 mybir.ActivationFunctionType.Sqrt, bias=eps_t[:])
            rstd = statp.tile([1, CTOK], f32, name=f"{L}_rstd", tag="stat")
            nc.vector.reciprocal(rstd[:], std[:])
            # broadcast along partitions via K=1 ones matmul
            mu_b = bcp.tile([P, CTOK], f32, name=f"{L}_mu_b", tag="bcast")
            rstd_b = bcp.tile([P, CTOK], f32, name=f"{L}_rstd_b", tag="bcast")
            for src, dst in ((mu, mu_b), (rstd, rstd_b)):
                pb = psB.tile([P, CTOK], f32, name=f"{L}_pb_{dst.name}", tag="psB")
                nc.tensor.matmul(pb[:], lhsT=ones1[:], rhs=src[:],
                                 start=True, stop=True)
                nc.scalar.copy(dst[:], pb[:])

            hs, ls = [], []
            for ot in range(n_ot):
                yh, yl = yhs[ot], yls[ot]
                t1 = lnt.tile([P, CTOK], f32, name=f"{L}_t1_{ot}", tag="lnt")
                nc.vector.tensor_sub(t1[:], yh[:], mu_b[:])
                nc.vector.tensor_add(t1[:], t1[:], yl[:])
                nc.vector.tensor_mul(t1[:], t1[:], rstd_b[:])
                g32 = lnt.tile([P, CTOK], f32, name=f"{L}_g32_{ot}", tag="lnt")
                nc.scalar.activation(g32[:], t1[:],
                                     mybir.ActivationFunctionType.Gelu,
                                     bias=be[:, ot, :], scale=g[:, ot, :])
                h = pool.tile([P, CTOK], bf16, name=f"{L}_h{ot}", tag=f"{L}a")
                l = pool.tile([P, CTOK], bf16, name=f"{L}_l{ot}", tag=f"{L}a")
                nc.scalar.copy(h[:], g32[:])
                nc.vector.tensor_sub(l[:], g32[:], h[:])
                hs.append(h)
                ls.append(l)
            return [(hs[i][:], ls[i][:]) for i in range(n_ot)]

        def logits_layer(L, rhs_pairs, lgp):
            """Linear into padded [128, CTOK] fp32 sbuf (adds bias)."""
            b = load_vec(f"{L}_b", P)
            res = lgp.tile([P, CTOK], f32, name=f"{L}_lg")

            def epi(ot, psum):
                nc.scalar.activation(res[:], psum,
                                     mybir.ActivationFunctionType.Identity,
                                     bias=b[:, 0, :])

            split_linear(L, rhs_pairs, epi)
            return res

        # ---------------- layer graph ----------------
        stp = top.enter_context(tc.tile_pool(name="stp", bufs=1))
        st = load_acts(stp, "st", ext["st_h"], ext["st_l"], S_DIM)

        lgp = top.enter_context(tc.tile_pool(name="lgp", bufs=1))

        t2sp = top.enter_context(tc.tile_pool(name="t2sp", bufs=2 * S_DIM // P + 2))
        with tc.tile_pool(name="tep", bufs=1) as tep:
            te = load_acts(tep, "te", ext["te_h"], ext["te_l"], T_DIM)
            t2s = plain_split_layer("t2s", te, t2sp, gelu=False)
            with tc.tile_pool(name="es1p", bufs=2 * (2 * S_DIM) // P + 8) as es1p:
                es1 = plain_split_layer("es1", te, es1p, gelu=True)
                lg_es = logits_layer("es2", es1, lgp)

        with tc.tile_pool(name="cap1p", bufs=2 * (2 * S_DIM) // P + 8) as cap1p:
            cap1 = ln_gelu_layer("cap1", st, cap1p)
            with tc.tile_pool(name="cap2p", bufs=2 * S_DIM // P + 8) as cap2p:
                cap2 = ln_gelu_layer("cap2", cap1, cap2p)
                lg_cap = logits_layer("cap3", cap2, lgp)

        with tc.tile_pool(name="gap1p", bufs=2 * T_DIM // P + 8) as gap1p:
            gap1 = ln_gelu_layer("gap1", st + t2s, gap1p)
            lg_gap = logits_layer("gap2", gap1, lgp)

        # ---------------- token-major epilogue ----------------
        mm_ctx.close()  # release matmul-phase PSUM banks
        tokp = top.enter_context(tc.tile_pool(name="tokp", bufs=40))
        psT = top.enter_context(tc.tile_pool(name="psT", bufs=2, space="PSUM"))
        gp = top.enter_context(tc.tile_pool(name="gp", bufs=4))
        outp = top.enter_context(tc.tile_pool(name="outp", bufs=2))

        def softmax_tok(lg, tt, scale):
            """transpose chunk tt of [128, CTOK] logits -> [P, 8] probs"""
            pst = psT.tile([P, P], f32, name=f"tr_{lg.name}_{tt}", tag="psT")
            nc.tensor.transpose(out=pst[:], in_=lg[:, tt * P:(tt + 1) * P],
                                identity=ident[:])
            lt = tokp.tile([P, E], f32, name=f"lt_{lg.name}_{tt}", tag="tok")
            nc.vector.tensor_copy(out=lt[:], in_=pst[:, 0:E])
            m = tokp.tile([P, 1], f32, name=f"m_{lg.name}_{tt}", tag="tok1")
            nc.vector.reduce_max(out=m[:], in_=lt[:], axis=mybir.AxisListType.X)
            d = tokp.tile([P, E], f32, name=f"d_{lg.name}_{tt}", tag="tok")
            nc.vector.tensor_scalar(d[:], lt[:], m[:], None,
                                    op0=mybir.AluOpType.subtract)
            e = tokp.tile([P, E], f32, name=f"e_{lg.name}_{tt}", tag="tok")
            den = tokp.tile([P, 1], f32, name=f"den_{lg.name}_{tt}", tag="tok1")
            nc.scalar.activation(e[:], d[:], mybir.ActivationFunctionType.Exp,
                                 scale=scale, accum_out=den[:])
            r = tokp.tile([P, 1], f32, name=f"r_{lg.name}_{tt}", tag="tok1")
            nc.vector.reciprocal(r[:], den[:])
            p = tokp.tile([P, E], f32, name=f"p_{lg.name}_{tt}", tag="tok")
            nc.vector.tensor_scalar(p[:], e[:], r[:], None,
                                    op0=mybir.AluOpType.mult)
            return p

        for tt in range(NTT):
            p_cap = softmax_tok(lg_cap, tt, inv_temp)
            p_gap = softmax_tok(lg_gap, tt, 1.0)
            p_es = softmax_tok(lg_es, tt, 1.0)
            comb = tokp.tile([P, E], f32, name=f"comb_{tt}", tag="tok")
            nc.vector.tensor_scalar(comb[:], p_cap[:], 0.4, None,
                                    op0=mybir.AluOpType.mult)
            tmp = tokp.tile([P, E], f32, name=f"cmb2_{tt}", tag="tok")
            nc.vector.tensor_scalar(tmp[:], p_gap[:], 0.3, None,
                                    op0=mybir.AluOpType.mult)
            nc.vector.tensor_add(comb[:], comb[:], tmp[:])
            nc.vector.tensor_scalar(tmp[:], p_es[:], 0.3, None,
                                    op0=mybir.AluOpType.mult)
            nc.vector.tensor_add(comb[:], comb[:], tmp[:])

            vals = tokp.tile([P, 8], f32, name=f"vals_{tt}", tag="tok")
            nc.vector.max(out=vals[:], in_=comb[:])
            idx = tokp.tile([P, 8], u32, name=f"idx_{tt}", tag="tok")
            nc.vector.max_index(out=idx[:], in_max=vals[:], in_values=comb[:])

            den2 = tokp.tile([P, 1], f32, name=f"den2_{tt}", tag="tok1")
            nc.vector.tensor_add(den2[:], vals[:, 0:1], vals[:, 1:2])
            nc.vector.tensor_scalar(den2[:], den2[:], 1e-8, None,
                                    op0=mybir.AluOpType.add)
            rden = tokp.tile([P, 1], f32, name=f"rden_{tt}", tag="tok1")
            nc.vector.reciprocal(rden[:], den2[:])
            w1 = tokp.tile([P, 1], f32, name=f"w1_{tt}", tag="tok1")
            w2 = tokp.tile([P, 1], f32, name=f"w2_{tt}", tag="tok1")
            nc.vector.tensor_mul(w1[:], vals[:, 0:1], rden[:])
            nc.vector.tensor_mul(w2[:], vals[:, 1:2], rden[:])

            gs = []
            for j, wj in ((0, w1), (1, w2)):
                idf = tokp.tile([P, 1], f32, name=f"idf{j}_{tt}", tag="tok1")
                nc.vector.tensor_copy(out=idf[:], in_=idx[:, j:j + 1])
                off_f = tokp.tile([P, 1], f32, name=f"offf{j}_{tt}", tag="tok1")
                nc.vector.tensor_scalar(off_f[:], idf[:], float(CTOK), iota_sb[:],
                                        op0=mybir.AluOpType.mult,
                                        op1=mybir.AluOpType.add)
                nc.vector.tensor_scalar(off_f[:], off_f[:], float(tt * P), None,
                                        op0=mybir.AluOpType.add)
                off_i = tokp.tile([P, 1], i32, name=f"offi{j}_{tt}", tag="tok1")
                nc.vector.tensor_copy(out=off_i[:], in_=off_f[:])
                gt = gp.tile([P, T_DIM], f32, name=f"g{j}_{tt}", tag="gather")
                nc.gpsimd.indirect_dma_start(
                    out=gt[:], out_offset=None, in_=ext["eo"][:],
                    in_offset=bass.IndirectOffsetOnAxis(ap=off_i[:, 0:1], axis=0))
                gs.append((gt, wj))

            o1 = outp.tile([P, T_DIM], f32, name=f"o1_{tt}", tag="outb")
            nc.scalar.activation(o1[:], gs[0][0][:],
                                 mybir.ActivationFunctionType.Copy,
                                 scale=gs[0][1][:])
            o2 = outp.tile([P, T_DIM], f32, name=f"o2_{tt}", tag="outb")
            nc.vector.tensor_scalar(o2[:], gs[1][0][:], gs[1][1][:], None,
                                    op0=mybir.AluOpType.mult)
            nc.vector.tensor_add(o1[:], o1[:], o2[:])
            nc.sync.dma_start(out=out_ext[tt * P:(tt + 1) * P, :], in_=o1[:])

    nc.compile()
    return nc


def _split(a):
    h = a.astype(bf)
    l = (a - h.astype(np.float32)).astype(bf)
    return h, l


def _prep_inputs(student_hidden, expert_outputs, params):
    """host-side prep: shard + transpose + hi/lo split + weight layout"""
    p = {k: np.asarray(v, dtype=np.float32) for k, v in params.items()}
    sh = np.ascontiguousarray(np.asarray(student_hidden, np.float32).reshape(NTOK, S_DIM))
    eo = np.asarray(expert_outputs, np.float32).reshape(E, NTOK, T_DIM)

    wmap = {
        "t2s": ("t2s_w", "t2s_b"), "es1": ("es_w1", "es_b1"), "es2": ("es_w2", "es_b2"),
        "cap1": ("cap_w1", "cap_b1"), "cap2": ("cap_w2", "cap_b2"), "cap3": ("cap_w3", "cap_b3"),
        "gap1": ("gap_w1", "gap_b1"), "gap2": ("gap_w2", "gap_b2"),
    }
    shared = {}
    for L, (wk, bk) in wmap.items():
        ind, outd = LAYERS[L]
        w = p[wk]  # [out, in] torch convention
        wT = np.ascontiguousarray(w.T)  # [in, out_real]
        b = p[bk]
        if wT.shape[1] < outd:  # pad logits layers to 128 outputs
            wT = np.concatenate(
                [wT, np.zeros((ind, outd - wT.shape[1]), np.float32)], axis=1)
            b = np.concatenate([b, np.zeros(outd - b.shape[0], np.float32)])
        wh, wl = _split(wT)
        shared[f"{L}_wh"] = np.ascontiguousarray(wh)
        shared[f"{L}_wl"] = np.ascontiguousarray(wl)
        shared[f"{L}_b"] = np.ascontiguousarray(b.reshape(outd, 1))
        if L in LN_LAYERS:
            shared[f"{L}_g"] = np.ascontiguousarray(p[wk.replace("_w", "_g")].reshape(outd, 1))
            ben = {"cap1": "cap_be1", "cap2": "cap_be2", "gap1": "gap_be1"}[L]
            shared[f"{L}_be"] = np.ascontiguousarray(p[ben].reshape(outd, 1))
    shared["iota"] = np.arange(P, dtype=np.float32).reshape(P, 1)

    in_maps = []
    for c in range(NCORES):
        cs = slice(c * CTOK, (c + 1) * CTOK)
        m = dict(shared)
        sth, stl = _split(np.ascontiguousarray(sh[cs].T))
        teh, tel = _split(np.ascontiguousarray(eo[0, cs].T))
        m["st_h"], m["st_l"] = sth, stl
        m["te_h"], m["te_l"] = teh, tel
        m["eo"] = np.ascontiguousarray(eo[:, cs, :]).reshape(E * CTOK, T_DIM)
        in_maps.append(m)
    return in_maps, float(1.0 / p["temp"].reshape(-1)[0])


_CACHE = {}


def kernel(student_hidden, expert_outputs, params):
    in_maps, inv_temp = _prep_inputs(student_hidden, expert_outputs, params)
    key = ("nc", inv_temp)
    if key not in _CACHE:
        _CACHE[key] = _build(inv_temp)
    nc = _CACHE[key]
    trace = bool(int(os.environ.get("KERNEL_TRACE", "0")))
    res = run_bass_kernel_spmd(nc, in_maps, core_ids=list(range(NCORES)),
                               trace=trace)
    if trace:
        kernel.last_exec_time_ns = res.exec_time_ns
        kernel.last_results = res
    out = np.concatenate([res.results[c]["out"] for c in range(NCORES)], axis=0)
    return out.reshape(2, NTOK // 2, T_DIM)


kernel.last_exec_time_ns = None
kernel.last_results = None


# revision 8
# speedup vs baseline: 1.0119x; 1.0119x over previous
"""AdaptiveExpertRouter Trainium2 kernel (8 NeuronCores, data-parallel over tokens).

Per-core pipeline (512 tokens, feature-major activations [feat_part, tok_free]):
  - every Linear is a 3-term bf16 hi/lo split matmul (x@W = xh@Wh + xh@Wl + xl@Wh)
    accumulated in fp32 PSUM -> ~7e-6 relative error, needed so top-2 expert
    selection matches the fp32 reference bit-for-bit on realistic score gaps
  - LayerNorm stats via ones-row matmuls (mean from bf16 hi/lo, E[x^2] from an
    fp32r Square), per-token broadcast back via K=1 ones matmul
  - logits transposed to token-major with PE transpose, softmax + top-2 via the
    DVE max8/max_index ops, then an indirect-DMA gather of the 2 selected
    expert rows per token and an fp32 weighted combine.
"""

import os
import sys
from contextlib import ExitStack

for _p in ("/opt/trn_rl_repo", "/root/.axon_site/_ro/trn_rl_repo"):
    if os.path.isdir(_p) and _p not in sys.path:
        sys.path.append(_p)

import numpy as np
import ml_dtypes

import concourse.bass as bass
import concourse.mybir as mybir
import concourse.bacc as bacc
from concourse.tile import TileContext
from concourse.bass_utils import run_bass_kernel_spmd
from concourse.masks import make_identity

P = 128
NCORES = 8
NTOK = 4096          # B*S
CTOK = NTOK // NCORES  # 512 tokens per core
NTT = CTOK // P      # 4 token tiles per core
S_DIM = 1024
T_DIM = 4096
E = 8

f32 = mybir.dt.float32
f32r = mybir.dt.float32r
bf16 = mybir.dt.bfloat16
i32 = mybir.dt.int32
u32 = mybir.dt.uint32

bf = ml_dtypes.bfloat16

# layer name -> (in_dim, out_dim, has_ln, has_gelu)
LAYERS = {
    "t2s":  (T_DIM, S_DIM),
    "es1":  (T_DIM, 2 * S_DIM),
    "es2":  (2 * S_DIM, P),      # logits padded 8 -> 128
    "cap1": (S_DIM, 2 * S_DIM),
    "cap2": (2 * S_DIM, S_DIM),
    "cap3": (S_DIM, P),          # logits padded
    "gap1": (2 * S_DIM, T_DIM),
    "gap2": (T_DIM, P),          # logits padded
}
LN_LAYERS = {"cap1", "cap2", "gap1"}


def _build(inv_temp: float):
    nc = bacc.Bacc("TRN2", target_bir_lowering=False, debug=False,
                   num_devices=NCORES)

    ext = {}
    ext["st_h"] = nc.dram_tensor("st_h", [S_DIM, CTOK], bf16, kind="ExternalInput")
    ext["st_l"] = nc.dram_tensor("st_l", [S_DIM, CTOK], bf16, kind="ExternalInput")
    ext["te_h"] = nc.dram_tensor("te_h", [T_DIM, CTOK], bf16, kind="ExternalInput")
    ext["te_l"] = nc.dram_tensor("te_l", [T_DIM, CTOK], bf16, kind="ExternalInput")
    for L, (ind, outd) in LAYERS.items():
        ext[f"{L}_wh"] = nc.dram_tensor(f"{L}_wh", [ind, outd], bf16, kind="ExternalInput")
        ext[f"{L}_wl"] = nc.dram_tensor(f"{L}_wl", [ind, outd], bf16, kind="ExternalInput")
        ext[f"{L}_b"] = nc.dram_tensor(f"{L}_b", [outd, 1], f32, kind="ExternalInput")
        if L in LN_LAYERS:
            ext[f"{L}_g"] = nc.dram_tensor(f"{L}_g", [outd, 1], f32, kind="ExternalInput")
            ext[f"{L}_be"] = nc.dram_tensor(f"{L}_be", [outd, 1], f32, kind="ExternalInput")
    ext["iota"] = nc.dram_tensor("iota", [P, 1], f32, kind="ExternalInput")
    ext["eo"] = nc.dram_tensor("eo", [E * CTOK, T_DIM], f32, kind="ExternalInput")
    out_ext = nc.dram_tensor("out", [CTOK, T_DIM], f32, kind="ExternalOutput")

    with TileContext(nc) as tc, ExitStack() as top:
        const = top.enter_context(tc.tile_pool(name="const", bufs=1))
        ident = const.tile([P, P], f32, name="ident")
        make_identity(nc, ident)
        iota_sb = const.tile([P, 1], f32, name="iota_sb")
        nc.sync.dma_start(out=iota_sb[:], in_=ext["iota"][:])
        eps_t = const.tile([1, 1], f32, name="eps_t")
        nc.vector.memset(eps_t[:], 1e-5)
        ones1 = const.tile([1, P], f32, name="ones1")
        nc.vector.memset(ones1[:], 1.0)
        # 1/D ones rows for stats matmuls (exact powers of two)
        ones_bf = {}
        ones_fr = {}
        for D in (S_DIM, 2 * S_DIM, T_DIM):
            tb = const.tile([P, 1], bf16, name=f"ones_bf_{D}")
            nc.vector.memset(tb[:], 1.0 / D)
            ones_bf[D] = tb
            t0 = const.tile([P, 1], f32, name=f"ones_f_{D}")
            nc.vector.memset(t0[:], 1.0 / D)
            tr = const.tile([P, 1], f32r, name=f"ones_fr_{D}")
            nc.vector.tensor_copy(out=tr[:], in_=t0[:])
            ones_fr[D] = tr

        biasp = top.enter_context(tc.tile_pool(name="biasp", bufs=1))
        wpool = top.enter_context(tc.tile_pool(name="wpool", bufs=6))
        lnt = top.enter_context(tc.tile_pool(name="lnt", bufs=8))
        statp = top.enter_context(tc.tile_pool(name="statp", bufs=4))
        bcp = top.enter_context(tc.tile_pool(name="bcp", bufs=2))
        mm_ctx = ExitStack()
        psA = mm_ctx.enter_context(tc.tile_pool(name="psA", bufs=5, space="PSUM"))
        psS = mm_ctx.enter_context(tc.tile_pool(name="psS", bufs=2, space="PSUM"))
        psB = mm_ctx.enter_context(tc.tile_pool(name="psB", bufs=1, space="PSUM"))

        def load_vec(name, outd):
            """[outd,1] fp32 dram -> sbuf tile"""
            t = biasp.tile([P, outd // P, 1], f32, name=f"{name}_sb")
            nc.sync.dma_start(
                out=t[:], in_=ext[name][:].rearrange("(ot p) one -> p ot one", p=P))
            return t

        def load_acts(pool, nm, src_h, src_l, dim):
            kt = dim // P
            h = pool.tile([P, kt, CTOK], bf16, name=f"{nm}_h")
            l = pool.tile([P, kt, CTOK], bf16, name=f"{nm}_l")
            nc.sync.dma_start(out=h[:], in_=src_h[:].rearrange("(kt p) n -> p kt n", p=P))
            nc.sync.dma_start(out=l[:], in_=src_l[:].rearrange("(kt p) n -> p kt n", p=P))
            return [(h[:, k, :], l[:, k, :]) for k in range(kt)]

        def split_linear(L, rhs_pairs, epilogue):
            """Emit the 3-term split matmuls for layer L; epilogue(ot, psum_ap)
            is called once per 128-row output tile."""
            ind, outd = LAYERS[L]
            nkt = ind // P
            assert len(rhs_pairs) == nkt
            n_ot = outd // P
            for og in range(0, n_ot, 4):
                ots = list(range(og, min(og + 4, n_ot)))
                ps = {}
                for ot in ots:
                    ps[ot] = psA.tile([P, CTOK], f32, name=f"{L}_ps{ot}", tag="psA")
                for kt in range(nkt):
                    cs = slice(ots[0] * P, (ots[-1] + 1) * P)
                    wh = wpool.tile([P, len(ots) * P], bf16, name=f"{L}_wh{og}_{kt}", tag="wblk")
                    wl = wpool.tile([P, len(ots) * P], bf16, name=f"{L}_wl{og}_{kt}", tag="wblk")
                    nc.sync.dma_start(out=wh[:], in_=ext[f"{L}_wh"][kt * P:(kt + 1) * P, cs])
                    nc.sync.dma_start(out=wl[:], in_=ext[f"{L}_wl"][kt * P:(kt + 1) * P, cs])
                    xh, xl = rhs_pairs[kt]
                    first = kt == 0
                    last = kt == nkt - 1
                    for j, ot in enumerate(ots):
                        sl = slice(j * P, (j + 1) * P)
                        nc.tensor.matmul(ps[ot][:], lhsT=wh[:, sl], rhs=xh,
                                         start=first, stop=False)
                        nc.tensor.matmul(ps[ot][:], lhsT=wl[:, sl], rhs=xh,
                                         start=False, stop=False)
                        nc.tensor.matmul(ps[ot][:], lhsT=wh[:, sl], rhs=xl,
                                         start=False, stop=last)
                for ot in ots:
                    epilogue(ot, ps[ot][:])

        def plain_split_layer(L, rhs_pairs, pool, gelu):
            """Linear (+bias) [+gelu], output split to bf16 hi/lo pairs."""
            _, outd = LAYERS[L]
            n_ot = outd // P
            b = load_vec(f"{L}_b", outd)
            hs, ls = [], []

            def epi(ot, psum):
                ba = b[:, ot, :]
                h = pool.tile([P, CTOK], bf16, name=f"{L}_h{ot}", tag=f"{L}a")
                l = pool.tile([P, CTOK], bf16, name=f"{L}_l{ot}", tag=f"{L}a")
                if gelu:
                    g32 = lnt.tile([P, CTOK], f32, name=f"{L}_g32_{ot}", tag="lnt")
                    nc.scalar.activation(g32[:], psum,
                                         mybir.ActivationFunctionType.Gelu, bias=ba)
                    nc.scalar.copy(h[:], g32[:])
                    nc.vector.tensor_sub(l[:], g32[:], h[:])
                else:
                    nc.scalar.activation(h[:], psum,
                                         mybir.ActivationFunctionType.Identity, bias=ba)
                    d1 = lnt.tile([P, CTOK], f32, name=f"{L}_d1_{ot}", tag="lnt")
                    nc.vector.tensor_sub(d1[:], psum, h[:])
                    nc.vector.tensor_scalar(l[:], d1[:], ba, None,
                                            op0=mybir.AluOpType.add)
                hs.append(h)
                ls.append(l)

            split_linear(L, rhs_pairs, epi)
            return [(hs[i][:], ls[i][:]) for i in range(n_ot)]

        def ln_gelu_layer(L, rhs_pairs, pool):
            """Linear + bias + LayerNorm(g,be) + exact GELU, split output."""
            _, outd = LAYERS[L]
            n_ot = outd // P
            b = load_vec(f"{L}_b", outd)
            g = load_vec(f"{L}_g", outd)
            be = load_vec(f"{L}_be", outd)
            mu_ps = psS.tile([1, CTOK], f32, name=f"{L}_mu", tag="psS")
            m2_ps = psS.tile([1, CTOK], f32, name=f"{L}_m2", tag="psS")
            yhs, yls = [], []

            def epi(ot, psum):
                ba = b[:, ot, :]
                yh = pool.tile([P, CTOK], bf16, name=f"{L}_yh{ot}", tag=f"{L}a")
                yl = pool.tile([P, CTOK], bf16, name=f"{L}_yl{ot}", tag=f"{L}a")
                nc.scalar.activation(yh[:], psum,
                                     mybir.ActivationFunctionType.Identity, bias=ba)
                d1 = lnt.tile([P, CTOK], f32, name=f"{L}_d1_{ot}", tag="lnt")
                nc.vector.tensor_sub(d1[:], psum, yh[:])
                nc.vector.tensor_scalar(yl[:], d1[:], ba, None,
                                        op0=mybir.AluOpType.add)
                sq = lnt.tile([P, CTOK], f32r, name=f"{L}_sq_{ot}", tag="lnt")
                nc.scalar.activation(sq[:], psum,
                                     mybir.ActivationFunctionType.Square, bias=ba)
                first = ot == 0
                last = ot == n_ot - 1
                nc.tensor.matmul(mu_ps[:], lhsT=ones_bf[outd][:], rhs=yh[:],
                                 start=first, stop=False)
                nc.tensor.matmul(mu_ps[:], lhsT=ones_bf[outd][:], rhs=yl[:],
                                 start=False, stop=last)
                nc.tensor.matmul(m2_ps[:], lhsT=ones_fr[outd][:], rhs=sq[:],
                                 start=first, stop=last)
                yhs.append(yh)
                yls.append(yl)

            split_linear(L, rhs_pairs, epi)

            # finalize stats: var = E[y^2] - mu^2 ; rstd = 1/sqrt(var+eps)
            mu = statp.tile([1, CTOK], f32, name=f"{L}_mu_sb", tag="stat")
            nc.vector.tensor_copy(out=mu[:], in_=mu_ps[:])
            var = statp.tile([1, CTOK], f32, name=f"{L}_var", tag="stat")
            nc.vector.tensor_mul(var[:], mu[:], mu[:])
            nc.vector.tensor_sub(var[:], m2_ps[:], var[:])
            std = statp.tile([1, CTOK], f32, name=f"{L}_std", tag="stat")
            nc.scalar.activation(std[:], var[:],
                                 mybir.ActivationFunctionType.Sqrt, bias=eps_t[:])
            rstd = statp.tile([1, CTOK], f32, name=f"{L}_rstd", tag="stat")
            nc.vector.reciprocal(rstd[:], std[:])
            # broadcast along partitions via K=1 ones matmul
            mu_b = bcp.tile([P, CTOK], f32, name=f"{L}_mu_b", tag="bcast")
            rstd_b = bcp.tile([P, CTOK], f32, name=f"{L}_rstd_b", tag="bcast")
            for src, dst in ((mu, mu_b), (rstd, rstd_b)):
                pb = psB.tile([P, CTOK], f32, name=f"{L}_pb_{dst.name}", tag="psB")
                nc.tensor.matmul(pb[:], lhsT=ones1[:], rhs=src[:],
                                 start=True, stop=True)
                nc.scalar.copy(dst[:], pb[:])

            hs, ls = [], []
            for ot in range(n_ot):
                yh, yl = yhs[ot], yls[ot]
                t1 = lnt.tile([P, CTOK], f32, name=f"{L}_t1_{ot}", tag="lnt")
                nc.vector.tensor_sub(t1[:], yh[:], mu_b[:])
                nc.vector.tensor_add(t1[:], t1[:], yl[:])
                nc.vector.tensor_mul(t1[:], t1[:], rstd_b[:])
                g32 = lnt.tile([P, CTOK], f32, name=f"{L}_g32_{ot}", tag="lnt")
                nc.scalar.activation(g32[:], t1[:],
                                     mybir.ActivationFunctionType.Gelu,
                                     bias=be[:, ot, :], scale=g[:, ot, :])
                h = pool.tile([P, CTOK], bf16, name=f"{L}_h{ot}", tag=f"{L}a")
                l = pool.tile([P, CTOK], bf16, name=f"{L}_l{ot}", tag=f"{L}a")
                nc.scalar.copy(h[:], g32[:])
                nc.vector.tensor_sub(l[:], g32[:], h[:])
                hs.append(h)
                ls.append(l)
            return [(hs[i][:], ls[i][:]) for i in range(n_ot)]

        def logits_layer(L, rhs_pairs, lgp):
            """Linear into padded [128, CTOK] fp32 sbuf (adds bias)."""
            b = load_vec(f"{L}_b", P)
            res = lgp.tile([P, CTOK], f32, name=f"{L}_lg")

            def epi(ot, psum):
                nc.scalar.activation(res[:], psum,
                                     mybir.ActivationFunctionType.Identity,
                                     bias=b[:, 0, :])

            split_linear(L, rhs_pairs, epi)
            return res

        # ---------------- layer graph ----------------
        stp = top.enter_context(tc.tile_pool(name="stp", bufs=1))
        st = load_acts(stp, "st", ext["st_h"], ext["st_l"], S_DIM)

        lgp = top.enter_context(tc.tile_pool(name="lgp", bufs=1))

        t2sp = top.enter_context(tc.tile_pool(name="t2sp", bufs=2 * S_DIM // P + 2))
        with tc.tile_pool(name="tep", bufs=1) as tep:
            te = load_acts(tep, "te", ext["te_h"], ext["te_l"], T_DIM)
            t2s = plain_split_layer("t2s", te, t2sp, gelu=False)
            with tc.tile_pool(name="es1p", bufs=2 * (2 * S_DIM) // P + 8) as es1p:
                es1 = plain_split_layer("es1", te, es1p, gelu=True)
                lg_es = logits_layer("es2", es1, lgp)

        with tc.tile_pool(name="cap1p", bufs=2 * (2 * S_DIM) // P + 8) as cap1p:
            cap1 = ln_gelu_layer("cap1", st, cap1p)
            with tc.tile_pool(name="cap2p", bufs=2 * S_DIM // P + 8) as cap2p:
                cap2 = ln_gelu_layer("cap2", cap1, cap2p)
                lg_cap = logits_layer("cap3", cap2, lgp)

        with tc.tile_pool(name="gap1p", bufs=2 * T_DIM // P + 8) as gap1p:
            gap1 = ln_gelu_layer("gap1", st + t2s, gap1p)
            lg_gap = logits_layer("gap2", gap1, lgp)

        # ---------------- token-major epilogue ----------------
        mm_ctx.close()  # release matmul-phase PSUM banks
        tokp = top.enter_context(tc.tile_pool(name="tokp", bufs=40))
        psT = top.enter_context(tc.tile_pool(name="psT", bufs=2, space="PSUM"))
        gp = top.enter_context(tc.tile_pool(name="gp", bufs=4))
        outp = top.enter_context(tc.tile_pool(name="outp", bufs=2))

        def softmax_tok(lg, tt, scale):
            """transpose chunk tt of [128, CTOK] logits -> [P, 8] probs"""
            pst = psT.tile([P, P], f32, name=f"tr_{lg.name}_{tt}", tag="psT")
            nc.tensor.transpose(out=pst[:], in_=lg[:, tt * P:(tt + 1) * P],
                                identity=ident[:])
            lt = tokp.tile([P, E], f32, name=f"lt_{lg.name}_{tt}", tag="tok")
            nc.vector.tensor_copy(out=lt[:], in_=pst[:, 0:E])
            m = tokp.tile([P, 1], f32, name=f"m_{lg.name}_{tt}", tag="tok1")
            nc.vector.reduce_max(out=m[:], in_=lt[:], axis=mybir.AxisListType.X)
            d = tokp.tile([P, E], f32, name=f"d_{lg.name}_{tt}", tag="tok")
            nc.vector.tensor_scalar(d[:], lt[:], m[:], None,
                                    op0=mybir.AluOpType.subtract)
            e = tokp.tile([P, E], f32, name=f"e_{lg.name}_{tt}", tag="tok")
            den = tokp.tile([P, 1], f32, name=f"den_{lg.name}_{tt}", tag="tok1")
            nc.scalar.activation(e[:], d[:], mybir.ActivationFunctionType.Exp,
                                 scale=scale, accum_out=den[:])
            r = tokp.tile([P, 1], f32, name=f"r_{lg.name}_{tt}", tag="tok1")
            nc.vector.reciprocal(r[:], den[:])
            p = tokp.tile([P, E], f32, name=f"p_{lg.name}_{tt}", tag="tok")
            nc.vector.tensor_scalar(p[:], e[:], r[:], None,
                                    op0=mybir.AluOpType.mult)
            return p

        for tt in range(NTT):
            p_cap = softmax_tok(lg_cap, tt, inv_temp)
            p_gap = softmax_tok(lg_gap, tt, 1.0)
            p_es = softmax_tok(lg_es, tt, 1.0)
            comb = tokp.tile([P, E], f32, name=f"comb_{tt}", tag="tok")
            nc.vector.tensor_scalar(comb[:], p_cap[:], 0.4, None,
                                    op0=mybir.AluOpType.mult)
            tmp = tokp.tile([P, E], f32, name=f"cmb2_{tt}", tag="tok")
            nc.vector.tensor_scalar(tmp[:], p_gap[:], 0.3, None,
                                    op0=mybir.AluOpType.mult)
            nc.vector.tensor_add(comb[:], comb[:], tmp[:])
            nc.vector.tensor_scalar(tmp[:], p_es[:], 0.3, None,
                                    op0=mybir.AluOpType.mult)
            nc.vector.tensor_add(comb[:], comb[:], tmp[:])

            vals = tokp.tile([P, 8], f32, name=f"vals_{tt}", tag="tok")
            nc.vector.max(out=vals[:], in_=comb[:])
            idx = tokp.tile([P, 8], u32, name=f"idx_{tt}", tag="tok")
            nc.vector.max_index(out=idx[:], in_max=vals[:], in_values=comb[:])

            den2 = tokp.tile([P, 1], f32, name=f"den2_{tt}", tag="tok1")
            nc.vector.tensor_add(den2[:], vals[:, 0:1], vals[:, 1:2])
            nc.vector.tensor_scalar(den2[:], den2[:], 1e-8, None,
                                    op0=mybir.AluOpType.add)
            rden = tokp.tile([P, 1], f32, name=f"rden_{tt}", tag="tok1")
            nc.vector.reciprocal(rden[:], den2[:])
            w1 = tokp.tile([P, 1], f32, name=f"w1_{tt}", tag="tok1")
            w2 = tokp.tile([P, 1], f32, name=f"w2_{tt}", tag="tok1")
            nc.vector.tensor_mul(w1[:], vals[:, 0:1], rden[:])
            nc.vector.tensor_mul(w2[:], vals[:, 1:2], rden[:])

            gs = []
            for j, wj in ((0, w1), (1, w2)):
                idf = tokp.tile([P, 1], f32, name=f"idf{j}_{tt}", tag="tok1")
                nc.vector.tensor_copy(out=idf[:], in_=idx[:, j:j + 1])
                off_f = tokp.tile([P, 1], f32, name=f"offf{j}_{tt}", tag="tok1")
                nc.vector.tensor_scalar(off_f[:], idf[:], float(CTOK), iota_sb[:],
                                        op0=mybir.AluOpType.mult,
                                        op1=mybir.AluOpType.add)
                nc.vector.tensor_scalar(off_f[:], off_f[:], float(tt * P), None,
                                        op0=mybir.AluOpType.add)
                off_i = tokp.tile([P, 1], i32, name=f"offi{j}_{tt}", tag="tok1")
                nc.vector.tensor_copy(out=off_i[:], in_=off_f[:])
                gt = gp.tile([P, T_DIM], f32, name=f"g{j}_{tt}", tag="gather")
                nc.gpsimd.indirect_dma_start(
                    out=gt[:], out_offset=None, in_=ext["eo"][:],
                    in_offset=bass.IndirectOffsetOnAxis(ap=off_i[:, 0:1], axis=0))
                gs.append((gt, wj))

            o1 = outp.tile([P, T_DIM], f32, name=f"o1_{tt}", tag="outb")
            nc.scalar.activation(o1[:], gs[0][0][:],
                                 mybir.ActivationFunctionType.Copy,
                                 scale=gs[0][1][:])
            o2 = outp.tile([P, T_DIM], f32, name=f"o2_{tt}", tag="outb")
            nc.vector.tensor_scalar(o2[:], gs[1][0][:], gs[1][1][:], None,
                                    op0=mybir.AluOpType.mult)
            nc.vector.tensor_add(o1[:], o1[:], o2[:])
            nc.sync.dma_start(out=out_ext[tt * P:(tt + 1) * P, :], in_=o1[:])

    nc.compile()
    return nc


def _split(a):
    h = a.astype(bf)
    l = (a - h.astype(np.float32)).astype(bf)
    return h, l


def _prep_inputs(student_hidden, expert_outputs, params):
    """host-side prep: shard + transpose + hi/lo split + weight layout"""
    p = {k: np.asarray(v, dtype=np.float32) for k, v in params.items()}
    sh = np.ascontiguousarray(np.asarray(student_hidden, np.float32).reshape(NTOK, S_DIM))
    eo = np.asarray(expert_outputs, np.float32).reshape(E, NTOK, T_DIM)

    wmap = {
        "t2s": ("t2s_w", "t2s_b"), "es1": ("es_w1", "es_b1"), "es2": ("es_w2", "es_b2"),
        "cap1": ("cap_w1", "cap_b1"), "cap2": ("cap_w2", "cap_b2"), "cap3": ("cap_w3", "cap_b3"),
        "gap1": ("gap_w1", "gap_b1"), "gap2": ("gap_w2", "gap_b2"),
    }
    shared = {}
    for L, (wk, bk) in wmap.items():
        ind, outd = LAYERS[L]
        w = p[wk]  # [out, in] torch convention
        wT = np.ascontiguousarray(w.T)  # [in, out_real]
        b = p[bk]
        if wT.shape[1] < outd:  # pad logits layers to 128 outputs
            wT = np.concatenate(
                [wT, np.zeros((ind, outd - wT.shape[1]), np.float32)], axis=1)
            b = np.concatenate([b, np.zeros(outd - b.shape[0], np.float32)])
        wh, wl = _split(wT)
        shared[f"{L}_wh"] = np.ascontiguousarray(wh)
        shared[f"{L}_wl"] = np.ascontiguousarray(wl)
        shared[f"{L}_b"] = np.ascontiguousarray(b.reshape(outd, 1))
        if L in LN_LAYERS:
            shared[f"{L}_g"] = np.ascontiguousarray(p[wk.replace("_w", "_g")].reshape(outd, 1))
            ben = {"cap1": "cap_be1", "cap2": "cap_be2", "gap1": "gap_be1"}[L]
            shared[f"{L}_be"] = np.ascontiguousarray(p[ben].reshape(outd, 1))
    shared["iota"] = np.arange(P, dtype=np.float32).reshape(P, 1)

    in_maps = []
    for c in range(NCORES):
        cs = slice(c * CTOK, (c + 1) * CTOK)
        m = dict(shared)
        sth, stl = _split(np.ascontiguousarray(sh[cs].T))
        teh, tel = _split(np.ascontiguousarray(eo[0, cs].T))
        m["st_h"], m["st_l"] = sth, stl
        m["te_h"], m["te_l"] = teh, tel
        m["eo"] = np.ascontiguousarray(eo[:, cs, :]).reshape(E * CTOK, T_DIM)
        in_maps.append(m)
    return in_maps, float(1.0 / p["temp"].reshape(-1)[0])


_CACHE = {}


def kernel(student_hidden, expert_outputs, params):
    in_maps, inv_temp = _prep_inputs(student_hidden, expert_outputs, params)
    key = ("nc", inv_temp)
    if key not in _CACHE:
        _CACHE[key] = _build(inv_temp)
    nc = _CACHE[key]
    trace = bool(int(os.environ.get("KERNEL_TRACE", "0")))
    tmpdir = os.environ.get("KERNEL_TRACE_DIR") or None
    res = run_bass_kernel_spmd(nc, in_maps, core_ids=list(range(NCORES)),
                               trace=trace, tmpdir=tmpdir)
    if trace:
        kernel.last_exec_time_ns = res.exec_time_ns
        kernel.last_results = res
    out = np.concatenate([res.results[c]["out"] for c in range(NCORES)], axis=0)
    return out.reshape(2, NTOK // 2, T_DIM)


kernel.last_exec_time_ns = None
kernel.last_results = None
